# revision 18
# baseline (speedup 1.0000x reference)
"""PodDeepONet2 Trainium2 kernel.

Reference computation (all fp32):
  out_b_{e,x,y} = branch_mlp(noise)          # (256, 128) each, dims 64->32x4->128, tanh
  out_t         = trunk_mlp(coordinates)     # (32768, 384), dims 2->128x4->384, tanh
  out_bias      = means_mlp(coordinates)     # (32768, 3),  dims 2->128x4->3, tanh
  g_f = (out_b_f @ out_t[:, f*128:(f+1)*128].T + out_bias[:, f][None, :]).reshape(-1)

Sharding: coordinate dim M=32768 split 8 ways (4096 per core); branch MLPs
replicated (tiny, the three run stacked as one 96-wide MLP). Host concatenates
the per-core column blocks.

On-chip everything runs transposed (features on partitions, batch in the free
dim): h_T = W.T @ x_T via matmul(psum, lhsT=W, rhs=x_T). Matmul operands are
float32r (fp32 PE fast path, 1 col/cycle vs 4; ~1e-4 rounding).

Key algebra: with h4/hm4 the last hidden layers of the two trunk MLPs,
  g_f = out_b_f @ (W4_f.T h4 + bt4_f 1^T).T + 1 (wm4_f.T hm4 + bm4_f)
      = (W4_f out_b_f).T @ h4 + (wm4_f 1^T).T @ hm4 + (out_b_f.T bt4_f + bm4_f) 1^T
so the trunk/means output layers fold into the GEMM:
  c_f   = W4_f @ out_b_f            (128x256, once per core, on PE)
  lhsT2 = wm4_f broadcast to 128 cols (host-built rank-1 stationary)
  d_f   = out_b_f.T @ bt4_f + bm4_f (256-vector, once; applied as the
          eviction instruction's per-partition scalar add)
Each GEMM psum accumulates two K=128 matmuls (c_f side + means side); the
eviction adds d_f. No trunk-L4 or means-L5 eviction traffic at all.
"""

import sys

import numpy as np

if "/opt/trn_rl_repo" not in sys.path:
    sys.path.insert(0, "/opt/trn_rl_repo")

import concourse.mybir as mybir
from concourse import bacc, bass_utils
from concourse.tile import TileContext

FP = mybir.dt.float32
FR = mybir.dt.float32r
N = 256            # noise batch
M = 32768          # coordinate points
NF = 64            # noise features
P = 128            # modes per field
NCORES = 8
ML = M // NCORES   # 4096 coords per core
CH = 1024          # trunk chunk width (2-bank psum tiles)
NCH = ML // CH
HF = 512           # matmul free-dim slice (one fp32 PSUM bank)

Tanh = mybir.ActivationFunctionType.Tanh
Ident = mybir.ActivationFunctionType.Identity

# packed f32r-matmul-operand layout: name -> (rows, cols); offsets cumulative
WPACK = [
    ("wt0", 2, 128), ("wm0", 2, 128), ("wt1", 128, 128), ("wm1", 128, 128),
    ("wt2", 128, 128), ("wm2", 128, 128), ("wt3", 128, 128), ("wm3", 128, 128),
    ("wt4t_0", 128, 128), ("wt4t_1", 128, 128), ("wt4t_2", 128, 128),
    ("wm4b_0", 128, 128), ("wm4b_1", 128, 128), ("wm4b_2", 128, 128),
    ("wb0", NF, 96), ("wb1", 96, 96), ("wb2", 96, 96), ("wb3", 96, 96),
    ("wb4", 96, 3 * P),
    ("noise_t", NF, N),
    ("bt4f_0", 128, 128), ("bt4f_1", 128, 128), ("bt4f_2", 128, 128),
]
WCOLS = sum(w for _, _, w in WPACK)

# packed fp32 per-partition bias columns (ACT bias / DVE scalar operands)
BPACK = (
    [f"bt{i}" for i in range(4)]
    + [f"bm{i}" for i in range(4)]
    + [f"bb{i}" for i in range(4)]
    + [f"bb4_{f}" for f in range(3)]
    + [f"bm4b_{f}" for f in range(3)]
)


def _program(nc, tc, coords, wpack, bpack, g):
    with (
        tc.tile_pool(name="const", bufs=1) as cpool,
        tc.tile_pool(name="act", bufs=2) as apool,
        tc.tile_pool(name="gout", bufs=4) as gpool,
        tc.tile_pool(name="psum", bufs=1, space="PSUM") as ppool,
    ):
        # ---- load + round constants (staged: chain weights first) ----
        CSPLIT = 1024  # interleaved (wt_i, wm_i) pairs fill the first 1024 cols
        w32 = cpool.tile([128, WCOLS], FP, name="w32")
        wfr = cpool.tile([128, WCOLS], FR, name="wfr")
        nc.sync.dma_start(out=w32[:, 0:CSPLIT], in_=wpack[:, 0:CSPLIT])
        for i in range(4):  # per-layer-pair casts so L0 unblocks asap
            nc.vector.tensor_copy(
                wfr[:, i * 256 : (i + 1) * 256], w32[:, i * 256 : (i + 1) * 256]
            )
        nc.sync.dma_start(out=w32[:, CSPLIT:], in_=wpack[:, CSPLIT:])
        nc.scalar.copy(wfr[:, CSPLIT:], w32[:, CSPLIT:])
        bsb = cpool.tile_from(bpack, name="bsb")
        w = {}
        off = 0
        for name, r, cwidth in WPACK:
            w[name] = wfr[0:r, off : off + cwidth]
            off += cwidth
        b = {name: bsb[:, i : i + 1] for i, name in enumerate(BPACK)}

        def ps_big(p=128, f=CH):
            return ppool.tile([p, f], FP, name="ps", bufs=3)

        def ps_small(p=128, f=HF):
            return ppool.tile([p, f], FP, name="psg", bufs=2)

        # ---- stacked branch MLP (one-time) ----
        h = w["noise_t"]
        for i in range(4):
            ps = ps_small(96, N)
            nc.tensor.matmul(ps, w[f"wb{i}"], h, start=True, stop=True)
            h = apool.tile([96, N], FR, name="h_br")
            nc.scalar.activation(h, ps, Tanh, bias=b[f"bb{i}"][0:96, :])
        outb = []
        for f in range(3):
            ps = ps_small(P, N)
            nc.tensor.matmul(
                ps, w["wb4"][:, f * P : (f + 1) * P], h, start=True, stop=True
            )
            ob = cpool.tile([P, N], FR, name=f"outb_{f}")
            nc.vector.tensor_scalar_add(ob, ps, b[f"bb4_{f}"])
            outb.append(ob)

        # ---- fold trunk L4 / means L5 into GEMM operands (one-time) ----
        cf, df = [], {}
        for f in range(3):
            ps = ps_small(128, N)
            nc.tensor.matmul(ps, w[f"wt4t_{f}"], outb[f], start=True, stop=True)
            c = cpool.tile([128, N], FR, name=f"cf_{f}")
            nc.vector.tensor_copy(c, ps)
            cf.append(c)
            for nt in range(2):
                ps2 = ps_small(128, 128)
                nc.tensor.matmul(
                    ps2,
                    outb[f][:, nt * P : (nt + 1) * P],
                    w[f"bt4f_{f}"],
                    start=True,
                    stop=True,
                )
                d = cpool.tile([128, 1], FP, name=f"d_{f}_{nt}")
                nc.scalar.activation(d, ps2[:, 0:1], Ident, bias=b[f"bm4b_{f}"])
                df[f, nt] = d

        # ---- trunk chains + GEMM, chunks of CH columns ----
        for c in range(NCH):
            cs = c * CH
            co32 = apool.tile([2, CH], FP, name="co32")
            nc.sync.dma_start(out=co32, in_=coords[:, cs : cs + CH])
            co = apool.tile([2, CH], FR, name="co")
            if c % 2 == 0:
                nc.vector.tensor_copy(co, co32)
            else:
                nc.scalar.copy(co, co32)

            h, hm = co, co
            for i in range(4):
                pst = ps_big()
                for q in range(2):
                    nc.tensor.matmul(
                        pst[:, q * HF : (q + 1) * HF],
                        w[f"wt{i}"],
                        h[:, q * HF : (q + 1) * HF],
                        start=True,
                        stop=True,
                    )
                psm = ps_big()
                for q in range(2):
                    nc.tensor.matmul(
                        psm[:, q * HF : (q + 1) * HF],
                        w[f"wm{i}"],
                        hm[:, q * HF : (q + 1) * HF],
                        start=True,
                        stop=True,
                    )
                h = apool.tile([128, CH], FR, name="ht", bufs=4)
                nc.scalar.activation(h, pst, Tanh, bias=b[f"bt{i}"])
                hm = apool.tile([128, CH], FR, name="hm", bufs=4)
                nc.scalar.activation(hm, psm, Tanh, bias=b[f"bm{i}"])

            # GEMM: psum = c_f.T @ h4 + (wm4_f 1^T).T @ hm4; evict with +d_f
            for f in range(3):
                for nt in range(2):
                    gt = gpool.tile([128, CH], FP, name="gt", bufs=6)
                    for q in range(2):
                        ps = ps_small()
                        nc.tensor.matmul(
                            ps,
                            cf[f][:, nt * P : (nt + 1) * P],
                            h[:, q * HF : (q + 1) * HF],
                            start=True,
                            stop=False,
                        )
                        nc.tensor.matmul(
                            ps,
                            w[f"wm4b_{f}"],
                            hm[:, q * HF : (q + 1) * HF],
                            start=False,
                            stop=True,
                        )
                        nc.vector.tensor_scalar_add(
                            gt[:, q * HF : (q + 1) * HF], ps, df[f, nt]
                        )
                    nc.sync.dma_start(
                        out=g[f, nt * P : (nt + 1) * P, cs : cs + CH], in_=gt
                    )


_CACHE = {}


def _get_compiled():
    if "nc" not in _CACHE:
        nc = bacc.Bacc(
            "TRN2", target_bir_lowering=False, debug=False, num_devices=NCORES
        )
        coords = nc.dram_tensor("coords_t", [2, ML], FP, kind="ExternalInput").ap()
        wpack = nc.dram_tensor("wpack", [128, WCOLS], FP, kind="ExternalInput").ap()
        bpack = nc.dram_tensor(
            "bpack", [128, len(BPACK)], FP, kind="ExternalInput"
        ).ap()
        g = nc.dram_tensor("g", [3, N, ML], FP, kind="ExternalOutput").ap()
        with TileContext(nc) as tc:
            _program(nc, tc, coords, wpack, bpack, g)
        nc.compile()
        _CACHE["nc"] = nc
    return _CACHE["nc"]


def _as_np(x):
    return np.ascontiguousarray(np.asarray(x, dtype=np.float32))


def make_in_maps(
    noise,
    coordinates,
    branch_e_params,
    branch_x_params,
    branch_y_params,
    trunk_comp_params,
    trunk_means_params,
):
    branches = [branch_e_params, branch_x_params, branch_y_params]
    wt4 = _as_np(trunk_comp_params[4][0])      # (128, 384)
    bt4 = _as_np(trunk_comp_params[4][1])      # (384,)
    wm4 = _as_np(trunk_means_params[4][0])     # (128, 3)
    bm4 = _as_np(trunk_means_params[4][1])     # (3,)

    ws = {"noise_t": _as_np(noise).T}
    ws["wb0"] = np.concatenate([_as_np(p[0][0]) for p in branches], axis=1)
    for i in range(1, 4):
        wb = np.zeros((96, 96), np.float32)
        for j, p in enumerate(branches):
            wb[32 * j : 32 * (j + 1), 32 * j : 32 * (j + 1)] = _as_np(p[i][0])
        ws[f"wb{i}"] = wb
    wb4 = np.zeros((96, 384), np.float32)
    for j, p in enumerate(branches):
        wb4[32 * j : 32 * (j + 1), 128 * j : 128 * (j + 1)] = _as_np(p[4][0])
    ws["wb4"] = wb4
    for i in range(4):
        ws[f"wt{i}"] = _as_np(trunk_comp_params[i][0])
        ws[f"wm{i}"] = _as_np(trunk_means_params[i][0])
    for f in range(3):
        ws[f"wt4t_{f}"] = np.ascontiguousarray(wt4[:, f * 128 : (f + 1) * 128].T)
        ws[f"wm4b_{f}"] = np.tile(wm4[:, f : f + 1], (1, 128))
        btp = np.zeros((128, 128), np.float32)
        btp[:, 0] = bt4[f * 128 : (f + 1) * 128]
        ws[f"bt4f_{f}"] = btp

    wpack = np.zeros((128, WCOLS), np.float32)
    off = 0
    for name, r, cwidth in WPACK:
        wpack[0:r, off : off + cwidth] = ws[name].reshape(r, cwidth)
        off += cwidth

    bs = {}
    for i in range(4):
        bs[f"bt{i}"] = _as_np(trunk_comp_params[i][1])
        bs[f"bm{i}"] = _as_np(trunk_means_params[i][1])
        bs[f"bb{i}"] = np.concatenate([_as_np(p[i][1]) for p in branches])
    for f in range(3):
        bs[f"bb4_{f}"] = _as_np(branches[f][4][1])
        bs[f"bm4b_{f}"] = np.full(128, bm4[f], np.float32)
    bpack = np.zeros((128, len(BPACK)), np.float32)
    for i, name in enumerate(BPACK):
        v = bs[name]
        bpack[0 : v.shape[0], i] = v

    coords_t = _as_np(coordinates).T  # (2, 32768)
    in_maps = []
    for c in range(NCORES):
        in_maps.append(
            {
                "wpack": wpack,
                "bpack": bpack,
                "coords_t": np.ascontiguousarray(
                    coords_t[:, c * ML : (c + 1) * ML]
                ),
            }
        )
    return in_maps


def run(in_maps, **kwargs):
    nc = _get_compiled()
    return bass_utils.run_bass_kernel_spmd(
        nc, in_maps, core_ids=list(range(NCORES)), **kwargs
    )


def kernel(**inputs):
    in_maps = make_in_maps(**inputs)
    res = run(in_maps)
    gs = [r["g"] for r in res.results]  # each (3, 256, 4096)
    out = []
    for f in range(3):
        out.append(
            np.concatenate([gs[c][f] for c in range(NCORES)], axis=1)
            .reshape(-1)
            .astype(np.float32)
        )
    return tuple(out)


# revision 19
# speedup vs baseline: 1.0544x; 1.0544x over previous
"""PodDeepONet2 Trainium2 kernel.

Reference computation (all fp32):
  out_b_{e,x,y} = branch_mlp(noise)          # (256, 128) each, dims 64->32x4->128, tanh
  out_t         = trunk_mlp(coordinates)     # (32768, 384), dims 2->128x4->384, tanh
  out_bias      = means_mlp(coordinates)     # (32768, 3),  dims 2->128x4->3, tanh
  g_f = (out_b_f @ out_t[:, f*128:(f+1)*128].T + out_bias[:, f][None, :]).reshape(-1)

Sharding: coordinate dim M=32768 split 8 ways (4096 per core); branch MLPs
replicated (tiny, the three run stacked as one 96-wide MLP). Host concatenates
the per-core column blocks.

On-chip everything runs transposed (features on partitions, batch in the free
dim): h_T = W.T @ x_T via matmul(psum, lhsT=W, rhs=x_T). Matmul operands are
float32r (fp32 PE fast path, 1 col/cycle vs 4; ~1e-4 rounding).

Key algebra: with h4/hm4 the last hidden layers of the two trunk MLPs,
  g_f = out_b_f @ (W4_f.T h4 + bt4_f 1^T).T + 1 (wm4_f.T hm4 + bm4_f)
      = (W4_f out_b_f).T @ h4 + (wm4_f 1^T).T @ hm4 + (out_b_f.T bt4_f + bm4_f) 1^T
so the trunk/means output layers fold into the GEMM:
  c_f   = W4_f @ out_b_f            (128x256, once per core, on PE)
  lhsT2 = wm4_f broadcast to 128 cols (host-built rank-1 stationary)
  d_f   = out_b_f.T @ bt4_f + bm4_f (256-vector, once; applied as the
          eviction instruction's per-partition scalar add)
Each GEMM psum accumulates two K=128 matmuls (c_f side + means side); the
eviction adds d_f. No trunk-L4 or means-L5 eviction traffic at all.
"""

import sys

import numpy as np

if "/opt/trn_rl_repo" not in sys.path:
    sys.path.insert(0, "/opt/trn_rl_repo")

import concourse.mybir as mybir
from concourse import bacc, bass_utils
from concourse.tile import TileContext

FP = mybir.dt.float32
FR = mybir.dt.float32r
N = 256            # noise batch
M = 32768          # coordinate points
NF = 64            # noise features
P = 128            # modes per field
NCORES = 8
ML = M // NCORES   # 4096 coords per core
CH = 1024          # trunk chunk width (2-bank psum tiles)
NCH = ML // CH
HF = 512           # matmul free-dim slice (one fp32 PSUM bank)

Tanh = mybir.ActivationFunctionType.Tanh
Ident = mybir.ActivationFunctionType.Identity

# packed f32r-matmul-operand layout: name -> (rows, cols); offsets cumulative
WPACK = [
    ("wt0", 2, 128), ("wm0", 2, 128), ("wt1", 128, 128), ("wm1", 128, 128),
    ("wt2", 128, 128), ("wm2", 128, 128), ("wt3", 128, 128), ("wm3", 128, 128),
    ("wt4t_0", 128, 128), ("wt4t_1", 128, 128), ("wt4t_2", 128, 128),
    ("wm4b_0", 128, 128), ("wm4b_1", 128, 128), ("wm4b_2", 128, 128),
    ("wb0", NF, 96), ("wb1", 96, 96), ("wb2", 96, 96), ("wb3", 96, 96),
    ("wb4", 96, 3 * P),
    ("noise_t", NF, N),
    ("bt4f_0", 128, 128), ("bt4f_1", 128, 128), ("bt4f_2", 128, 128),
]
WCOLS = sum(w for _, _, w in WPACK)

# packed fp32 per-partition bias columns (ACT bias / DVE scalar operands)
BPACK = (
    [f"bt{i}" for i in range(4)]
    + [f"bm{i}" for i in range(4)]
    + [f"bb{i}" for i in range(4)]
    + [f"bb4_{f}" for f in range(3)]
    + [f"bm4b_{f}" for f in range(3)]
)


def _program(nc, tc, coords, wpack, bpack, g):
    with (
        tc.tile_pool(name="const", bufs=1) as cpool,
        tc.tile_pool(name="act", bufs=2) as apool,
        tc.tile_pool(name="gout", bufs=4) as gpool,
        tc.tile_pool(name="psum", bufs=1, space="PSUM") as ppool,
    ):
        # ---- load + round constants (staged: chain weights first) ----
        CSPLIT = 1024  # interleaved (wt_i, wm_i) pairs fill the first 1024 cols
        w32 = cpool.tile([128, WCOLS], FP, name="w32")
        wfr = cpool.tile([128, WCOLS], FR, name="wfr")
        nc.sync.dma_start(out=w32[:, 0:CSPLIT], in_=wpack[:, 0:CSPLIT])
        for i in range(4):  # per-layer-pair casts so L0 unblocks asap
            nc.vector.tensor_copy(
                wfr[:, i * 256 : (i + 1) * 256], w32[:, i * 256 : (i + 1) * 256]
            )
        nc.sync.dma_start(out=w32[:, CSPLIT:], in_=wpack[:, CSPLIT:])
        nc.scalar.copy(wfr[:, CSPLIT:], w32[:, CSPLIT:])
        bsb = cpool.tile_from(bpack, name="bsb")
        w = {}
        off = 0
        for name, r, cwidth in WPACK:
            w[name] = wfr[0:r, off : off + cwidth]
            off += cwidth
        b = {name: bsb[:, i : i + 1] for i, name in enumerate(BPACK)}

        # PE warm-up: dummy bf16 matmuls during the head's DMA wait so the
        # HAM clock-gate is at 8/8 when real matmuls arrive
        BF = mybir.dt.bfloat16
        wa = cpool.tile([128, 128], BF, name="warm_a")
        wb = cpool.tile([128, HF], BF, name="warm_b")
        nc.gpsimd.memset(wa, 0.0)
        nc.gpsimd.memset(wb, 0.0)
        for _ in range(16):
            pw = ppool.tile([128, HF], FP, name="psg", bufs=2)
            nc.tensor.matmul(pw, wa, wb, start=True, stop=True)

        def ps_big(p=128, f=CH):
            return ppool.tile([p, f], FP, name="ps", bufs=3)

        def ps_small(p=128, f=HF):
            return ppool.tile([p, f], FP, name="psg", bufs=2)

        # ---- stacked branch MLP (one-time) ----
        h = w["noise_t"]
        for i in range(4):
            ps = ps_small(96, N)
            nc.tensor.matmul(ps, w[f"wb{i}"], h, start=True, stop=True)
            h = apool.tile([96, N], FR, name="h_br")
            nc.scalar.activation(h, ps, Tanh, bias=b[f"bb{i}"][0:96, :])
        outb = []
        for f in range(3):
            ps = ps_small(P, N)
            nc.tensor.matmul(
                ps, w["wb4"][:, f * P : (f + 1) * P], h, start=True, stop=True
            )
            ob = cpool.tile([P, N], FR, name=f"outb_{f}")
            nc.vector.tensor_scalar_add(ob, ps, b[f"bb4_{f}"])
            outb.append(ob)

        # ---- fold trunk L4 / means L5 into GEMM operands (one-time) ----
        cf, df = [], {}
        for f in range(3):
            ps = ps_small(128, N)
            nc.tensor.matmul(ps, w[f"wt4t_{f}"], outb[f], start=True, stop=True)
            c = cpool.tile([128, N], FR, name=f"cf_{f}")
            nc.vector.tensor_copy(c, ps)
            cf.append(c)
            for nt in range(2):
                ps2 = ps_small(128, 128)
                nc.tensor.matmul(
                    ps2,
                    outb[f][:, nt * P : (nt + 1) * P],
                    w[f"bt4f_{f}"],
                    start=True,
                    stop=True,
                )
                d = cpool.tile([128, 1], FP, name=f"d_{f}_{nt}")
                nc.scalar.activation(d, ps2[:, 0:1], Ident, bias=b[f"bm4b_{f}"])
                df[f, nt] = d

        # ---- trunk chains + GEMM, chunks of CH columns ----
        for c in range(NCH):
            cs = c * CH
            co32 = apool.tile([2, CH], FP, name="co32")
            nc.sync.dma_start(out=co32, in_=coords[:, cs : cs + CH])
            co = apool.tile([2, CH], FR, name="co")
            if c % 2 == 0:
                nc.vector.tensor_copy(co, co32)
            else:
                nc.scalar.copy(co, co32)

            h, hm = co, co
            for i in range(4):
                pst = ps_big()
                for q in range(2):
                    nc.tensor.matmul(
                        pst[:, q * HF : (q + 1) * HF],
                        w[f"wt{i}"],
                        h[:, q * HF : (q + 1) * HF],
                        start=True,
                        stop=True,
                    )
                psm = ps_big()
                for q in range(2):
                    nc.tensor.matmul(
                        psm[:, q * HF : (q + 1) * HF],
                        w[f"wm{i}"],
                        hm[:, q * HF : (q + 1) * HF],
                        start=True,
                        stop=True,
                    )
                h = apool.tile([128, CH], FR, name="ht", bufs=4)
                nc.scalar.activation(h, pst, Tanh, bias=b[f"bt{i}"])
                hm = apool.tile([128, CH], FR, name="hm", bufs=4)
                nc.scalar.activation(hm, psm, Tanh, bias=b[f"bm{i}"])

            # GEMM: psum = c_f.T @ h4 + (wm4_f 1^T).T @ hm4; evict with +d_f
            for f in range(3):
                for nt in range(2):
                    gt = gpool.tile([128, CH], FP, name="gt", bufs=6)
                    for q in range(2):
                        ps = ps_small()
                        nc.tensor.matmul(
                            ps,
                            cf[f][:, nt * P : (nt + 1) * P],
                            h[:, q * HF : (q + 1) * HF],
                            start=True,
                            stop=False,
                        )
                        nc.tensor.matmul(
                            ps,
                            w[f"wm4b_{f}"],
                            hm[:, q * HF : (q + 1) * HF],
                            start=False,
                            stop=True,
                        )
                        nc.vector.tensor_scalar_add(
                            gt[:, q * HF : (q + 1) * HF], ps, df[f, nt]
                        )
                    nc.sync.dma_start(
                        out=g[f, nt * P : (nt + 1) * P, cs : cs + CH], in_=gt
                    )


_CACHE = {}


def _get_compiled():
    if "nc" not in _CACHE:
        nc = bacc.Bacc(
            "TRN2", target_bir_lowering=False, debug=False, num_devices=NCORES
        )
        coords = nc.dram_tensor("coords_t", [2, ML], FP, kind="ExternalInput").ap()
        wpack = nc.dram_tensor("wpack", [128, WCOLS], FP, kind="ExternalInput").ap()
        bpack = nc.dram_tensor(
            "bpack", [128, len(BPACK)], FP, kind="ExternalInput"
        ).ap()
        g = nc.dram_tensor("g", [3, N, ML], FP, kind="ExternalOutput").ap()
        with TileContext(nc) as tc:
            _program(nc, tc, coords, wpack, bpack, g)
        nc.compile()
        _CACHE["nc"] = nc
    return _CACHE["nc"]


def _as_np(x):
    return np.ascontiguousarray(np.asarray(x, dtype=np.float32))


def make_in_maps(
    noise,
    coordinates,
    branch_e_params,
    branch_x_params,
    branch_y_params,
    trunk_comp_params,
    trunk_means_params,
):
    branches = [branch_e_params, branch_x_params, branch_y_params]
    wt4 = _as_np(trunk_comp_params[4][0])      # (128, 384)
    bt4 = _as_np(trunk_comp_params[4][1])      # (384,)
    wm4 = _as_np(trunk_means_params[4][0])     # (128, 3)
    bm4 = _as_np(trunk_means_params[4][1])     # (3,)

    ws = {"noise_t": _as_np(noise).T}
    ws["wb0"] = np.concatenate([_as_np(p[0][0]) for p in branches], axis=1)
    for i in range(1, 4):
        wb = np.zeros((96, 96), np.float32)
        for j, p in enumerate(branches):
            wb[32 * j : 32 * (j + 1), 32 * j : 32 * (j + 1)] = _as_np(p[i][0])
        ws[f"wb{i}"] = wb
    wb4 = np.zeros((96, 384), np.float32)
    for j, p in enumerate(branches):
        wb4[32 * j : 32 * (j + 1), 128 * j : 128 * (j + 1)] = _as_np(p[4][0])
    ws["wb4"] = wb4
    for i in range(4):
        ws[f"wt{i}"] = _as_np(trunk_comp_params[i][0])
        ws[f"wm{i}"] = _as_np(trunk_means_params[i][0])
    for f in range(3):
        ws[f"wt4t_{f}"] = np.ascontiguousarray(wt4[:, f * 128 : (f + 1) * 128].T)
        ws[f"wm4b_{f}"] = np.tile(wm4[:, f : f + 1], (1, 128))
        btp = np.zeros((128, 128), np.float32)
        btp[:, 0] = bt4[f * 128 : (f + 1) * 128]
        ws[f"bt4f_{f}"] = btp

    wpack = np.zeros((128, WCOLS), np.float32)
    off = 0
    for name, r, cwidth in WPACK:
        wpack[0:r, off : off + cwidth] = ws[name].reshape(r, cwidth)
        off += cwidth

    bs = {}
    for i in range(4):
        bs[f"bt{i}"] = _as_np(trunk_comp_params[i][1])
        bs[f"bm{i}"] = _as_np(trunk_means_params[i][1])
        bs[f"bb{i}"] = np.concatenate([_as_np(p[i][1]) for p in branches])
    for f in range(3):
        bs[f"bb4_{f}"] = _as_np(branches[f][4][1])
        bs[f"bm4b_{f}"] = np.full(128, bm4[f], np.float32)
    bpack = np.zeros((128, len(BPACK)), np.float32)
    for i, name in enumerate(BPACK):
        v = bs[name]
        bpack[0 : v.shape[0], i] = v

    coords_t = _as_np(coordinates).T  # (2, 32768)
    in_maps = []
    for c in range(NCORES):
        in_maps.append(
            {
                "wpack": wpack,
                "bpack": bpack,
                "coords_t": np.ascontiguousarray(
                    coords_t[:, c * ML : (c + 1) * ML]
                ),
            }
        )
    return in_maps


def run(in_maps, **kwargs):
    nc = _get_compiled()
    return bass_utils.run_bass_kernel_spmd(
        nc, in_maps, core_ids=list(range(NCORES)), **kwargs
    )


def kernel(**inputs):
    in_maps = make_in_maps(**inputs)
    res = run(in_maps)
    gs = [r["g"] for r in res.results]  # each (3, 256, 4096)
    out = []
    for f in range(3):
        out.append(
            np.concatenate([gs[c][f] for c in range(NCORES)], axis=1)
            .reshape(-1)
            .astype(np.float32)
        )
    return tuple(out)
